# revision 1
# baseline (speedup 1.0000x reference)
"""AttentionPairBias kernel for 8 Trainium2 NeuronCores.

Sharding: data-parallel over query rows i (768 -> 8 x 96). Each core gets the
full s (to recompute k/v locally, which is cheaper than an all-gather) and its
own contiguous slice z[:, i0:i0+96] of the huge pair tensor (302 MB total, so
the z read is perfectly sharded). Each core produces output rows [i0, i0+96);
the host concatenates. No cross-core collective is needed.
"""

import numpy as np

B, N, H, DH, CZ = 1, 768, 16, 32, 128
D = H * DH
NC = 8
IB = N // NC  # 96 rows per core
EPS = 1e-5


def _build_fn():
    import jax
    import jax.numpy as jnp

    def ln(x, w, b):
        mu = jnp.mean(x, axis=-1, keepdims=True)
        var = jnp.var(x, axis=-1, keepdims=True)
        return (x - mu) * jax.lax.rsqrt(var + EPS) * w + b

    def fn(s, z_sl, norm_s_w, norm_s_b, q_w, q_b, k_w, v_w, g_w, zn_w, zn_b,
           z_w, o_w):
        # s: [1, N, D] full; z_sl: [1, IB, N, CZ] this core's row slice
        s_n = ln(s, norm_s_w, norm_s_b)
        s_n32 = s_n.astype(jnp.float32)
        k = (s_n32 @ k_w).reshape(B, N, H, DH)
        v = (s_n32 @ v_w).reshape(B, N, H, DH)
        # this core's query rows are a slice of the full normed s
        return s_n, k, v

    def blk(s_n_blk, k, v, q_w, q_b, g_w, z_sl, zn_w, zn_b, z_w, o_w):
        q = (s_n_blk @ q_w + q_b).reshape(B, IB, H, DH)
        scores = jnp.einsum("bihd,bjhd->bhij", q, k) * (DH ** -0.5)
        zb = ln(z_sl, zn_w, zn_b) @ z_w                  # [B, IB, N, H]
        zb = jnp.transpose(zb, (0, 3, 1, 2))             # [B, H, IB, N]
        a = jax.nn.softmax(scores + zb, axis=-1)
        o = jnp.einsum("bhij,bjhd->bihd", a, v).reshape(B, IB, D)
        g = jax.nn.sigmoid(s_n_blk @ g_w)
        return (o * g) @ o_w                             # [B, IB, D]

    def full(i0, s, z, norm_s_w, norm_s_b, q_w, q_b, k_w, v_w, g_w,
             zn_w, zn_b, z_w, o_w):
        z_sl = z
        s_n, k, v = fn(s, z_sl, norm_s_w, norm_s_b, q_w, q_b, k_w, v_w,
                       g_w, zn_w, zn_b, z_w, o_w)
        s_n_blk = jax.lax.dynamic_slice_in_dim(s_n, i0, IB, axis=1)
        return blk(s_n_blk, k, v, q_w, q_b, g_w, z_sl, zn_w, zn_b, z_w, o_w)

    return full


def kernel(**inputs):
    import jax

    full = _build_fn()
    devs = jax.devices()[:NC]
    jfull = jax.jit(full, static_argnums=0)

    z = inputs["z"]
    outs = []
    # dispatch all 8 cores asynchronously, then gather
    for d in range(NC):
        i0 = d * IB
        args = dict(inputs)
        args["z"] = np.ascontiguousarray(z[:, i0:i0 + IB])
        dargs = {k: jax.device_put(v, devs[d]) for k, v in args.items()}
        outs.append(jfull(i0, **dargs))
    res = [np.asarray(o) for o in outs]
    out = np.concatenate(res, axis=1).astype(np.float32)
    return out


if __name__ == "__main__":
    rng = np.random.default_rng(0)
    ins = {
        "s": rng.standard_normal((B, N, D), dtype=np.float32),
        "z": rng.standard_normal((B, N, N, CZ), dtype=np.float32),
        "norm_s_w": np.ones(D, np.float32),
        "norm_s_b": np.zeros(D, np.float32),
        "q_w": rng.standard_normal((D, D), dtype=np.float32) * 0.02,
        "q_b": rng.standard_normal(D, dtype=np.float32) * 0.02,
        "k_w": rng.standard_normal((D, D), dtype=np.float32) * 0.02,
        "v_w": rng.standard_normal((D, D), dtype=np.float32) * 0.02,
        "g_w": rng.standard_normal((D, D), dtype=np.float32) * 0.02,
        "zn_w": np.ones(CZ, np.float32),
        "zn_b": np.zeros(CZ, np.float32),
        "z_w": rng.standard_normal((CZ, H), dtype=np.float32) * 0.02,
        "o_w": rng.standard_normal((D, D), dtype=np.float32) * 0.02,
    }
    out = kernel(**ins)
    print(out.shape, out.dtype)

